# revision 1
# baseline (speedup 1.0000x reference)
"""CKConv via bandlimited-DFT decomposition (Trainium2, 8 cores).

The SIREN-generated kernel g[o,i,d] (flipped, causal) is smooth: after
splitting off a 32-tap head and a 32-tap tail corner with C2 smoothstep
masks, the middle's 4096-pt DFT lives in the first P=256 bins to ~4e-4
relative.  The causal conv then becomes three cheap pieces:

  1. short conv: taps d<32, dense PE GEMM (exact).
  2. middle: out_mid = IDFT_P( Ghat[f] * Xhat[f] ):
       - forward DFT GEMM: Xhat[c,f; b,i] = sum_tau x * cos/-sin
       - per-freq 32x32 complex channel mix (K=(c,i) -> M=(c',o))
       - inverse DFT GEMM over the P kept bins
     Frequencies are sharded 32/core; partial inverse outputs are summed
     on the host (linearity).
  3. tail corner: taps d>=2016 only affect the last 33 outputs -> host.

Per-core PE work ~14.5k rows vs ~66.5k for the dense Toeplitz baseline.
Core k: freqs [32k, 32k+32); short conv for batch k//2, time-half k%2.
"""

import numpy as np

import concourse.mybir as mybir
import concourse.tile as tile
from concourse import bacc
from concourse.bass_utils import run_bass_kernel_spmd

B, CIN, COUT, L, HID = 4, 32, 32, 2048, 32
OMEGA = 30.0
NCORES = 8
NF = 4096          # DFT length (covers linear conv exactly)
P = 256            # kept frequency bins
FPC = P // NCORES  # 32 freqs per core
WN = 32            # short-conv taps (head mask width)
WT = 32            # tail corner width
XIMW = 1056        # short-conv im2col width: 1024 + WN - 4 + pad
NWARM = 10

TRACE = False
LAST_EXEC_NS = None
LAST_RESULTS = None

_NC = None
_TABLES = None


def _build_nc():
    nc = bacc.Bacc(None, target_bir_lowering=False)
    f32 = mybir.dt.float32
    f16 = mybir.dt.float16

    xt_d = nc.dram_tensor("xt", [128, 16 * 128], f16, kind="ExternalInput")
    wfwd_d = nc.dram_tensor("wfwd", [128, 16, 64], f16, kind="ExternalInput")
    gpw_d = nc.dram_tensor("gpw", [64, FPC, 64], f16, kind="ExternalInput")
    winv_d = nc.dram_tensor("winv", [64, 16, 128], f16, kind="ExternalInput")
    xim_d = nc.dram_tensor("xim", [128, XIMW], f16, kind="ExternalInput")
    wsh_d = nc.dram_tensor("wsh", [128, WN // 4, 32], f16, kind="ExternalInput")
    omid_d = nc.dram_tensor("omid", [128, 2048], f32, kind="ExternalOutput")
    osh_d = nc.dram_tensor("osh", [32, 1024], f32, kind="ExternalOutput")

    with tile.TileContext(nc) as tc:
        with (
            tc.tile_pool(name="const", bufs=1) as cpool,
            tc.tile_pool(name="pswarm", bufs=1, space="PSUM") as pswarm,
            tc.tile_pool(name="pssmall", bufs=1, space="PSUM") as pssmall,
            tc.tile_pool(name="psshort", bufs=2, space="PSUM") as psshort,
            tc.tile_pool(name="psout", bufs=3, space="PSUM") as psout,
        ):
            # PE warmup on a zeroed dummy: lifts the p-state clock while
            # the input DMAs stream.
            dummy = cpool.tile([128, 512], mybir.dt.bfloat16)
            nc.vector.memset(dummy[:], 0.0)
            wps = pswarm.tile([128, 512], f32)
            for _ in range(NWARM):
                nc.tensor.matmul(
                    wps[:], dummy[:, 0:128], dummy[:], start=True, stop=True
                )

            # --- input DMAs (short-conv data first: it can start earliest)
            wsh = cpool.tile([128, WN // 4, 32], f16)
            xim = cpool.tile([128, XIMW], f16)
            winv = cpool.tile([64, 16, 128], f16)
            nc.sync.dma_start(out=xim[:], in_=xim_d[:])
            nc.sync.dma_start(out=wsh[:], in_=wsh_d[:])
            nc.sync.dma_start(out=winv[:], in_=winv_d[:])

            wfwd = cpool.tile([128, 16, 64], f16)
            xt = cpool.tile([128, 16 * 128], f16)
            gpw = cpool.tile([64, FPC, 64], f16)
            nc.scalar.dma_start(out=wfwd[:], in_=wfwd_d[:])
            nc.scalar.dma_start(out=xt[:], in_=xt_d[:])
            nc.scalar.dma_start(out=gpw[:], in_=gpw_d[:])

            # --- short conv: out[o, u] = sum_{beta,dj,i} g_short * x
            # xim[dj*32+i, c] = x[i, h*1024 - 32 + c - dj]; tap = 4j + dj
            # out col u reads xim col u + 32 - 4j.  The two sub-tiles are
            # emitted BETWEEN the freq-path stages so the PE runs them
            # while the DVE does the inter-stage repacks.
            osh_sb = cpool.tile([32, 1024], f32)

            def short_sub(sub):
                ps = psshort.tile([32, 512], f32)
                for j in range(WN // 4):
                    s = sub * 512 + 32 - 4 * j
                    nc.tensor.matmul(
                        ps[:],
                        wsh[:, j, :],
                        xim[:, s : s + 512],
                        start=(j == 0),
                        stop=(j == WN // 4 - 1),
                    )
                if sub == 0:
                    nc.scalar.copy(osh_sb[:, 0:512], ps[:])
                else:
                    nc.vector.tensor_copy(osh_sb[:, 512:1024], ps[:])

            short_sub(0)   # first PE work: gated only on xim/wsh (earliest DMAs)

            # --- forward DFT: PS_X[(c,fl), (b,i)] over 16 tau-tiles
            ps_x = pssmall.tile([64, 128], f32)
            for kt in range(16):
                nc.tensor.matmul(
                    ps_x[:],
                    wfwd[:, kt, :],
                    xt[:, kt * 128 : kt * 128 + 128],
                    start=(kt == 0),
                    stop=(kt == 15),
                )
            s1 = cpool.tile([64, 128], f16)
            nc.vector.tensor_copy(s1[:], ps_x[:])
            # 32x32 block transpose: [(c,fl),(b,i)] -> XT[(c,i),(b,fl)]
            xt2 = cpool.tile([64, 128], f16)
            nc.vector.transpose(xt2[:], s1[:])

            short_sub(1)   # PE filler while DVE repacks ps_x -> xt2
            nc.sync.dma_start(out=osh_d[:], in_=osh_sb[:])

            # --- pointwise complex channel mix, one call per local freq
            # PS_A[(c',o), b*32+fl] += gpw[:, fl, :].T @ XT[:, b*32+fl]
            ps_a = pssmall.tile([64, 128], f32)
            for fl in range(FPC):
                nc.tensor.matmul(
                    ps_a[:, fl : fl + 97 : 32],
                    gpw[:, fl, :],
                    xt2[:, fl : fl + 97 : 32],
                    start=True,
                    stop=True,
                )
            s2 = cpool.tile([64, 128], f16)
            nc.vector.tensor_copy(s2[:], ps_a[:])
            at = cpool.tile([64, 128], f16)
            nc.vector.transpose(at[:], s2[:])


            # --- inverse DFT partial: 16 t-tiles, 4 per psum bank
            for q in range(4):
                ps_o = psout.tile([128, 512], f32)
                for jj in range(4):
                    tt = 4 * q + jj
                    nc.tensor.matmul(
                        ps_o[:, jj * 128 : jj * 128 + 128],
                        winv[:, tt, :],
                        at[:],
                        start=True,
                        stop=True,
                    )
                stg = cpool.tile([128, 512], f32, tag=f"stg{q}")
                if q == 3:
                    # last bank: split copy+DMA across engine pairs so the
                    # end-of-kernel chain is half-length
                    nc.vector.tensor_copy(stg[:, 0:256], ps_o[:, 0:256])
                    nc.scalar.copy(stg[:, 256:512], ps_o[:, 256:512])
                    nc.sync.dma_start(
                        out=omid_d[:, 1536:1792], in_=stg[:, 0:256])
                    nc.gpsimd.dma_start(
                        out=omid_d[:, 1792:2048], in_=stg[:, 256:512])
                else:
                    if q % 2 == 0:
                        nc.vector.tensor_copy(stg[:], ps_o[:])
                    else:
                        nc.scalar.copy(stg[:], ps_o[:])
                    dma_eng = nc.sync if q % 2 == 0 else nc.gpsimd
                    dma_eng.dma_start(
                        out=omid_d[:, q * 512 : q * 512 + 512], in_=stg[:]
                    )

    nc.compile()
    return nc


def _gen_flipped_kernel(w1, b1, w2, b2, w3, b3):
    pos = np.linspace(-1.0, 1.0, L, dtype=np.float64)[::-1]
    h = np.sin(OMEGA * (w1.astype(np.float64)[:, 0][:, None] * pos[None, :]
                        + b1.astype(np.float64)[:, None]))
    h = np.sin(OMEGA * (w2.astype(np.float64) @ h + b2.astype(np.float64)[:, None]))
    k = w3.astype(np.float64) @ h + b3.astype(np.float64)[:, None]
    return k.reshape(COUT, CIN, L)


def _smoothstep(u):
    u = np.clip(u, 0.0, 1.0)
    return u * u * u * (10.0 - 15.0 * u + 6.0 * u * u)


def _dft_tables():
    """Input-independent cos/sin GEMM tables, per core."""
    global _TABLES
    if _TABLES is not None:
        return _TABLES
    tau = np.arange(L)
    t = np.arange(L)
    wfwd = np.zeros((NCORES, 128, 16, 64), dtype=np.float16)
    winv = np.zeros((NCORES, 64, 16, 128), dtype=np.float16)
    for k in range(NCORES):
        f = (k * FPC + np.arange(FPC)).astype(np.float64)
        ang_f = 2.0 * np.pi * np.outer(tau, f) / NF          # [L, FPC]
        cosf = np.cos(ang_f).reshape(16, 128, FPC)
        sinf = -np.sin(ang_f).reshape(16, 128, FPC)
        wfwd[k, :, :, 0:32] = cosf.transpose(1, 0, 2)
        wfwd[k, :, :, 32:64] = sinf.transpose(1, 0, 2)
        ang_t = 2.0 * np.pi * np.outer(f, t) / NF            # [FPC, L]
        cost = np.cos(ang_t).reshape(FPC, 16, 128)
        sint = -np.sin(ang_t).reshape(FPC, 16, 128)
        winv[k, 0:32] = cost
        winv[k, 32:64] = sint
    d = np.arange(L, dtype=np.float64)
    wn_mask = 1.0 - _smoothstep(d / WN)
    wt_mask = _smoothstep((d - (L - 1 - WT)) / WT)
    _TABLES = (wfwd, winv, wn_mask, wt_mask)
    return _TABLES


def kernel(x, w1, b1, w2, b2, w3, b3, bias):
    global _NC, LAST_EXEC_NS, LAST_RESULTS
    x = np.ascontiguousarray(np.asarray(x, dtype=np.float32))
    bias = np.asarray(bias, dtype=np.float32)
    wfwd, winv, wn_mask, wt_mask = _dft_tables()

    g = _gen_flipped_kernel(np.asarray(w1), np.asarray(b1), np.asarray(w2),
                            np.asarray(b2), np.asarray(w3), np.asarray(b3))
    g_short = g * wn_mask[None, None, :]
    g_tail = g * wt_mask[None, None, :]
    g_mid = g * (1.0 - wn_mask - wt_mask)[None, None, :]

    # pointwise weights: Ghat (with 2/NF scale folded; 1/NF at f=0)
    G = np.fft.rfft(g_mid.reshape(COUT * CIN, L), n=NF, axis=1)[:, :P]
    G = G.reshape(COUT, CIN, P)
    sf = np.full(P, 2.0 / NF)
    sf[0] = 1.0 / NF
    Gr = (G.real * sf).astype(np.float16)
    Gi = (G.imag * sf).astype(np.float16)
    gpw = np.zeros((NCORES, 64, FPC, 64), dtype=np.float16)
    for k in range(NCORES):
        fs = slice(k * FPC, (k + 1) * FPC)
        # K=(c,i) -> M=(c',o):  Are = Gr Xre - Gi Xim ; Aim = Gi Xre + Gr Xim
        gpw[k, 0:32, :, 0:32] = Gr[:, :, fs].transpose(1, 2, 0)
        gpw[k, 32:64, :, 0:32] = -Gi[:, :, fs].transpose(1, 2, 0)
        gpw[k, 0:32, :, 32:64] = Gi[:, :, fs].transpose(1, 2, 0)
        gpw[k, 32:64, :, 32:64] = Gr[:, :, fs].transpose(1, 2, 0)

    # short-conv weights: wsh[dj*32+i, j, o] = g_short[o, i, 4j+dj]
    gs = g_short[:, :, :WN].astype(np.float16)                 # [o, i, tap]
    wsh = np.ascontiguousarray(
        gs.reshape(COUT, CIN, WN // 4, 4).transpose(3, 1, 2, 0)
          .reshape(128, WN // 4, 32).astype(np.float16))

    xh = x.astype(np.float16)
    # xt[p, kt*128 + b*32+i] = x[b, i, kt*128+p]
    xt = np.ascontiguousarray(
        xh.reshape(B * CIN, 16, 128).transpose(2, 1, 0).reshape(128, 16 * 128))
    # xim[dj*32+i, c] = x[b, i, h*1024 - 32 + c - dj]
    xims = np.zeros((B, 2, 128, XIMW), dtype=np.float16)
    for h in range(2):
        t0 = h * 1024 - 32
        for dj in range(4):
            blk = slice(32 * dj, 32 * dj + 32)
            lo = t0 - dj            # x index at xim col 0
            c0 = max(0, -lo)        # first valid col
            src0 = lo + c0
            n = min(XIMW - c0, L - src0)
            xims[:, h, blk, c0 : c0 + n] = xh[:, :, src0 : src0 + n]

    if _NC is None:
        _NC = _build_nc()

    in_maps = []
    for k in range(NCORES):
        b, h = k // 2, k % 2
        in_maps.append({
            "xt": xt,
            "wfwd": np.ascontiguousarray(wfwd[k]),
            "gpw": np.ascontiguousarray(gpw[k]),
            "winv": np.ascontiguousarray(winv[k]),
            "xim": np.ascontiguousarray(xims[b, h]),
            "wsh": wsh,
        })

    res = run_bass_kernel_spmd(_NC, in_maps, core_ids=list(range(NCORES)),
                               trace=TRACE)
    LAST_RESULTS = res
    LAST_EXEC_NS = res.exec_time_ns

    # gather: sum per-core inverse partials, scatter short-conv halves
    out = np.zeros((B, COUT, L), dtype=np.float64)
    for k in range(NCORES):
        om = res.results[k]["omid"]          # [128, 2048] f32
        # omid[tloc, q*512 + jj*128 + (b*32+o)] = out_mid[b,o,(4q+jj)*128+tloc]
        om = om.reshape(128, 16, B, COUT).transpose(2, 3, 1, 0).reshape(
            B, COUT, L)
        out += om
        b, h = k // 2, k % 2
        out[b, :, h * 1024 : h * 1024 + 1024] += res.results[k]["osh"]

    # tail corner on host: taps d in [2016, 2047] touch only t >= 2016
    gt = g_tail[:, :, L - 1 - WT :]                        # [o, i, 33]
    xd = x.astype(np.float64)
    for dd in range(L - 1 - WT, L):
        out[:, :, dd:] += np.einsum(
            "oi,bit->bot", gt[:, :, dd - (L - 1 - WT)], xd[:, :, : L - dd])

    out += bias[None, :, None]
    return out.astype(np.float32)



# revision 4
# speedup vs baseline: 1.3623x; 1.3623x over previous
"""CKConv via bandlimited-DFT decomposition (Trainium2, 8 cores), v2.

The SIREN-generated kernel g[o,i,d] (flipped, causal) is smooth: after
splitting off a 32-tap head and a 32-tap tail corner with C2 smoothstep
masks, the middle's 4096-pt DFT lives in the first P=192 bins to ~7e-3
relative.  The causal conv then becomes three cheap pieces:

  1. short conv: taps d<32, dense PE GEMM (exact), phase-packed so all
     128 PE output columns are used: out[(r,o), u] for t = 4u+r, via 9
     accumulating matmuls of K=128 (shift-class q), N=256.
  2. middle: out_mid = IDFT_P( Ghat[f] * Xhat[f] ):
       - forward DFT GEMM over 16 tau-tiles (xt streamed in 2 DMA halves
         so the first 8 matmuls overlap the second half's DMA)
       - per-freq 32x32 complex channel mix (24 freqs/core)
       - inverse DFT with the mixed spectrum as the stationary operand:
         4 matmuls of N=512 producing omid[(b,o), t] directly.
     Frequencies are sharded 24/core; partial inverse outputs (f16) are
     summed on the host (linearity).
  3. tail corner: taps d>=2016 only affect the last 33 outputs -> host.

v2 vs v1: P 256->192, all outputs f16, inverse restructured 16->4
matmuls, short conv 16->9 matmuls with full M=128 packing, input DMAs
issued first and ordered by consumer, tensor-engine instruction count
kept under the 256-instruction IRAM block so the epilogue never fetches
a second block.  Core k: freqs [24k, 24k+24); short conv for batch k//2,
time-half k%2.
"""

import numpy as np

import concourse.mybir as mybir
import concourse.tile as tile
from concourse import bacc
from concourse.bass_utils import run_bass_kernel_spmd

B, CIN, COUT, L, HID = 4, 32, 32, 2048, 32
OMEGA = 30.0
NCORES = 8
NF = 4096          # DFT length (covers linear conv exactly)
P = 192            # kept frequency bins
FPC = P // NCORES  # 24 freqs per core
WN = 32            # short-conv taps (head mask width)
WT = 32            # tail corner width
NQ = 9             # short-conv shift-class blocks
XDW = 264          # decimated-x width: 256 u-cols + 8 lead
NWARM = 4

TRACE = False
LAST_EXEC_NS = None
LAST_RESULTS = None

_NC = None
_TABLES = None


def _build_nc():
    nc = bacc.Bacc(None, target_bir_lowering=False)
    f32 = mybir.dt.float32
    f16 = mybir.dt.float16

    xt_d = nc.dram_tensor("xt", [128, 16 * 128], f16, kind="ExternalInput")
    wfwd_d = nc.dram_tensor("wfwd", [128, 16, 64], f16, kind="ExternalInput")
    gpw_d = nc.dram_tensor("gpw", [64, FPC, 64], f16, kind="ExternalInput")
    winv_d = nc.dram_tensor("winv", [64, 2048], f16, kind="ExternalInput")
    xd_d = nc.dram_tensor("xd", [128, XDW], f16, kind="ExternalInput")
    wsh_d = nc.dram_tensor("wsh", [128, NQ, 128], f16, kind="ExternalInput")
    omid_d = nc.dram_tensor("omid", [128, 2048], f16, kind="ExternalOutput")
    osh_d = nc.dram_tensor("osh", [128, 256], f16, kind="ExternalOutput")

    with tile.TileContext(nc) as tc:
        with (
            tc.tile_pool(name="const", bufs=1) as cpool,
            tc.tile_pool(name="pswarm", bufs=1, space="PSUM") as pswarm,
            tc.tile_pool(name="psx", bufs=1, space="PSUM") as psx,
            tc.tile_pool(name="psa", bufs=1, space="PSUM") as psa,
            tc.tile_pool(name="pssh", bufs=1, space="PSUM") as pssh,
            tc.tile_pool(name="psout", bufs=4, space="PSUM") as psout,
        ):
            # --- input DMAs first: engine queue order == arrival order.
            # scalar: xt_lo, gpw, wsh ; sync: wfwd, xt_hi, winv, xd
            xt = cpool.tile([128, 16 * 128], f16)
            wfwd = cpool.tile([128, 16, 64], f16)
            gpw = cpool.tile([64, FPC, 64], f16)
            winv = cpool.tile([64, 2048], f16)
            xd = cpool.tile([128, XDW], f16)
            wsh = cpool.tile([128, NQ, 128], f16)

            nc.scalar.dma_start(out=xt[:, 0:1024], in_=xt_d[:, 0:1024])
            nc.sync.dma_start(out=wfwd[:], in_=wfwd_d[:])
            nc.sync.dma_start(out=xt[:, 1024:2048], in_=xt_d[:, 1024:2048])
            nc.scalar.dma_start(out=gpw[:], in_=gpw_d[:])
            nc.sync.dma_start(out=winv[:], in_=winv_d[:])
            nc.scalar.dma_start(out=wsh[:], in_=wsh_d[:])
            nc.sync.dma_start(out=xd[:], in_=xd_d[:])

            # --- PE warmup on a zeroed dummy: lifts the HAM clock gate
            # while the input DMAs stream.
            dummy = cpool.tile([128, 512], mybir.dt.bfloat16)
            nc.vector.memset(dummy[:], 0.0)
            # s2 is only partially written by the pointwise stage; zero it
            # so the block transpose reads defined data.
            s2 = cpool.tile([64, 4, 32], f16)
            nc.vector.memset(s2[:], 0.0)
            wps = pswarm.tile([128, 512], f32)
            for _ in range(NWARM):
                nc.tensor.matmul(
                    wps[:], dummy[:, 0:128], dummy[:], start=True, stop=True
                )

            # --- forward DFT: ps_x[(c,fl32), (b,i)] over 16 tau-tiles
            ps_x = psx.tile([64, 128], f32)
            for kt in range(16):
                nc.tensor.matmul(
                    ps_x[:],
                    wfwd[:, kt, :],
                    xt[:, kt * 128 : kt * 128 + 128],
                    start=(kt == 0),
                    stop=(kt == 15),
                )
            s1 = cpool.tile([64, 128], f16)
            nc.vector.tensor_copy(s1[:], ps_x[:])
            # 32x32 block transpose: [(c,fl),(b,i)] -> XT[(c,i),(b,fl)]
            xt2 = cpool.tile([64, 4, 32], f16)
            nc.vector.transpose(xt2[:], s1[:])

            # --- pointwise complex channel mix, one call per local freq
            # ps_a[(c',o), b*32+fl] = gpw[:, fl, :].T @ XT[:, b*32+fl]
            ps_a = psa.tile([64, 4, 32], f32)
            for fl in range(FPC):
                nc.tensor.matmul(
                    ps_a[:, :, fl],
                    gpw[:, fl, :],
                    xt2[:, :, fl],
                    start=True,
                    stop=True,
                )
            nc.vector.tensor_copy(s2[:, :, 0:FPC], ps_a[:, :, 0:FPC])
            at = cpool.tile([64, 128], f16)
            nc.vector.transpose(at[:], s2[:])

            # --- inverse DFT: omid[(b,o), t] = at.T @ winv, 4 big tiles.
            # at rows fl>=FPC are zero, so winv values there are dont-care.
            stg = [None] * 2
            for q in range(4):
                ps_o = psout.tile([128, 512], f32)
                nc.tensor.matmul(
                    ps_o[:], at[:], winv[:, q * 512 : q * 512 + 512],
                    start=True, stop=True,
                )
                if q % 2 == 0:
                    stg[q // 2] = cpool.tile(
                        [128, 1024], f16, name=f"stg{q // 2}", tag=f"stg{q // 2}")
                half = stg[q // 2]
                dst = half[:, (q % 2) * 512 : (q % 2) * 512 + 512]
                if q % 2 == 0:
                    nc.vector.tensor_copy(dst, ps_o[:])
                else:
                    nc.scalar.copy(dst, ps_o[:])
                    dma_eng = nc.sync if q == 1 else nc.gpsimd
                    dma_eng.dma_start(
                        out=omid_d[:, (q // 2) * 1024 : (q // 2) * 1024 + 1024],
                        in_=half[:],
                    )

            # --- short conv (off the critical path: fills PE slack while
            # the output casts/DMAs drain): 9 accumulating shift-blocks.
            ps_sh = pssh.tile([128, 256], f32)
            for qi in range(NQ):
                nc.tensor.matmul(
                    ps_sh[:],
                    wsh[:, qi, :],
                    xd[:, qi : qi + 256],
                    start=(qi == 0),
                    stop=(qi == NQ - 1),
                )
            osh_sb = cpool.tile([128, 256], f16)
            nc.scalar.copy(osh_sb[:], ps_sh[:])
            nc.scalar.dma_start(out=osh_d[:], in_=osh_sb[:])

    nc.compile()
    return nc


def _gen_flipped_kernel(w1, b1, w2, b2, w3, b3):
    pos = np.linspace(-1.0, 1.0, L, dtype=np.float64)[::-1]
    h = np.sin(OMEGA * (w1.astype(np.float64)[:, 0][:, None] * pos[None, :]
                        + b1.astype(np.float64)[:, None]))
    h = np.sin(OMEGA * (w2.astype(np.float64) @ h + b2.astype(np.float64)[:, None]))
    k = w3.astype(np.float64) @ h + b3.astype(np.float64)[:, None]
    return k.reshape(COUT, CIN, L)


def _smoothstep(u):
    u = np.clip(u, 0.0, 1.0)
    return u * u * u * (10.0 - 15.0 * u + 6.0 * u * u)


def _dft_tables():
    """Input-independent cos/sin GEMM tables, per core."""
    global _TABLES
    if _TABLES is not None:
        return _TABLES
    tau = np.arange(L)
    t = np.arange(L)
    wfwd = np.zeros((NCORES, 128, 16, 64), dtype=np.float16)
    winv = np.zeros((NCORES, 64, 2048), dtype=np.float16)
    for k in range(NCORES):
        f = (k * FPC + np.arange(FPC)).astype(np.float64)
        ang_f = 2.0 * np.pi * np.outer(tau, f) / NF          # [L, FPC]
        cosf = np.cos(ang_f).reshape(16, 128, FPC)
        sinf = -np.sin(ang_f).reshape(16, 128, FPC)
        wfwd[k, :, :, 0:FPC] = cosf.transpose(1, 0, 2)
        wfwd[k, :, :, 32:32 + FPC] = sinf.transpose(1, 0, 2)
        ang_t = 2.0 * np.pi * np.outer(f, t) / NF            # [FPC, L]
        winv[k, 0:FPC] = np.cos(ang_t)
        winv[k, 32:32 + FPC] = -np.sin(ang_t)
    d = np.arange(L, dtype=np.float64)
    wn_mask = 1.0 - _smoothstep(d / WN)
    wt_mask = _smoothstep((d - (L - 1 - WT)) / WT)
    _TABLES = (wfwd, winv, wn_mask, wt_mask)
    return _TABLES


def kernel(x, w1, b1, w2, b2, w3, b3, bias):
    global _NC, LAST_EXEC_NS, LAST_RESULTS
    x = np.ascontiguousarray(np.asarray(x, dtype=np.float32))
    bias = np.asarray(bias, dtype=np.float32)
    wfwd, winv, wn_mask, wt_mask = _dft_tables()

    g = _gen_flipped_kernel(np.asarray(w1), np.asarray(b1), np.asarray(w2),
                            np.asarray(b2), np.asarray(w3), np.asarray(b3))
    g_short = g * wn_mask[None, None, :]
    g_tail = g * wt_mask[None, None, :]
    g_mid = g * (1.0 - wn_mask - wt_mask)[None, None, :]

    # pointwise weights: Ghat (with 2/NF scale folded; 1/NF at f=0)
    G = np.fft.rfft(g_mid.reshape(COUT * CIN, L), n=NF, axis=1)[:, :P]
    G = G.reshape(COUT, CIN, P)
    sf = np.full(P, 2.0 / NF)
    sf[0] = 1.0 / NF
    Gr = (G.real * sf).astype(np.float16)
    Gi = (G.imag * sf).astype(np.float16)
    gpw = np.zeros((NCORES, 64, FPC, 64), dtype=np.float16)
    for k in range(NCORES):
        fs = slice(k * FPC, (k + 1) * FPC)
        # K=(c,i) -> M=(c',o):  Are = Gr Xre - Gi Xim ; Aim = Gi Xre + Gr Xim
        gpw[k, 0:32, :, 0:32] = Gr[:, :, fs].transpose(1, 2, 0)
        gpw[k, 32:64, :, 0:32] = -Gi[:, :, fs].transpose(1, 2, 0)
        gpw[k, 0:32, :, 32:64] = Gi[:, :, fs].transpose(1, 2, 0)
        gpw[k, 32:64, :, 32:64] = Gr[:, :, fs].transpose(1, 2, 0)

    # short-conv weights: wsh[(s,i), qi, (r*32+o)] = gs[o,i, r-4(qi-8)-s]
    gs = g_short[:, :, :WN]
    wsh = np.zeros((128, NQ, 128), dtype=np.float16)
    for qi in range(NQ):
        q = qi - 8
        for s in range(4):
            for r in range(4):
                tap = r - 4 * q - s
                if 0 <= tap < WN:
                    wsh[s * 32:(s + 1) * 32, qi, r * 32:(r + 1) * 32] = (
                        gs[:, :, tap].T.astype(np.float16))

    xh = x.astype(np.float16)
    # xt[p, kt*128 + b*32+i] = x[b, i, kt*128+p]
    xt = np.ascontiguousarray(
        xh.reshape(B * CIN, 16, 128).transpose(2, 1, 0).reshape(128, 16 * 128))
    # xd[s*32+i, col] = x[b, i, h*1024 + 4*(col-8) + s]   (decimated x)
    xds = np.zeros((B, 2, 128, XDW), dtype=np.float16)
    col = np.arange(XDW)
    for b in range(B):
        for h in range(2):
            for s in range(4):
                idx = h * 1024 + 4 * (col - 8) + s
                valid = (idx >= 0) & (idx < L)
                xds[b, h, s * 32:(s + 1) * 32, valid] = xh[b, :, idx[valid]]

    if _NC is None:
        _NC = _build_nc()

    in_maps = []
    for k in range(NCORES):
        b, h = k // 2, k % 2
        in_maps.append({
            "xt": xt,
            "wfwd": np.ascontiguousarray(wfwd[k]),
            "gpw": np.ascontiguousarray(gpw[k]),
            "winv": np.ascontiguousarray(winv[k]),
            "xd": np.ascontiguousarray(xds[b, h]),
            "wsh": wsh,
        })

    res = run_bass_kernel_spmd(_NC, in_maps, core_ids=list(range(NCORES)),
                               trace=TRACE)
    LAST_RESULTS = res
    LAST_EXEC_NS = res.exec_time_ns

    # gather: sum per-core inverse partials, scatter short-conv quarters
    out = np.zeros((B, COUT, L), dtype=np.float64)
    for k in range(NCORES):
        om = res.results[k]["omid"]          # [b*32+o, t] f16
        out += om.astype(np.float64).reshape(B, COUT, L)
        b, h = k // 2, k % 2
        osh = res.results[k]["osh"].astype(np.float64)   # [r*32+o, u]
        for r in range(4):
            out[b, :, h * 1024 + r:h * 1024 + 1024:4] += osh[r * 32:(r + 1) * 32, :]

    # tail corner on host: taps d in [2016, 2047] touch only t >= 2016
    gt = g_tail[:, :, L - 1 - WT:]                        # [o, i, 33]
    xdd = x.astype(np.float64)
    for dd in range(L - 1 - WT, L):
        out[:, :, dd:] += np.einsum(
            "oi,bit->bot", gt[:, :, dd - (L - 1 - WT)], xdd[:, :, :L - dd])

    out += bias[None, :, None]
    return out.astype(np.float32)
